# revision 4
# baseline (speedup 1.0000x reference)
"""Trainium2 Bass kernel for nn_BoundaryPredictor2 (ragged_sequence).

Data-parallel over batch: one NeuronCore per batch row (B=8, 8 cores).

Per-core pipeline (one batch row, L=4096 tokens, D=1024):
  cos_sim(j) = n_j^T (Wq^T Wk) n_{j+1}  with n_l = h_l / |h_l|
    -> M = Wq^T @ Wk once per core (PE); H transposed chunk-wise on PE;
       P^T = M^T H^T; G[j] = sum_e P^T[e,j] H^T[e,j+1] (DVE mult + PE
       ones-reduce).  Normalization folded into row scalars.
  boundary decision: sigmoid(logit(p)+logit(u)) > 0.5  <=>  p + u > 1
    (p = (1-cos)/2, u = clipped noise) -- no transcendentals needed.
  segment-mean pooling: inclusive prefix sums along tokens (DVE scan over a
    second transposed pass), boundary-position lists via iota+sparse_gather,
    column gathers via ap_gather, per-segment 1/len scale fused into the
    PSUM->SBUF copy after PE transpose-back.
"""

import math

import numpy as np

L = 4096
D = 1024
B = 8
NCORES = 8
LCH = 512            # L-chunk for projection / gathers
NLC = L // LCH       # 8
NTT = L // 128       # 32 token tiles
NDC = D // 128       # 8 d-chunks
PRIOR = 0.2

_CACHE = {}


def build_nc():
    import concourse.bacc as bacc
    import concourse.mybir as mybir
    from concourse import masks, tile

    F32 = mybir.dt.float32
    I16 = mybir.dt.int16
    U32 = mybir.dt.uint32
    Alu = mybir.AluOpType
    Act = mybir.ActivationFunctionType
    X = mybir.AxisListType.X

    nc = bacc.Bacc("TRN2", target_bir_lowering=False, debug=False,
                   num_devices=NCORES)

    h_ext = nc.dram_tensor("h", [L, D], F32, kind="ExternalInput").ap()
    nz_ext = nc.dram_tensor("nz", [1, L], F32, kind="ExternalInput").ap()
    mk_ext = nc.dram_tensor("mk", [1, L], F32, kind="ExternalInput").ap()
    wq_ext = nc.dram_tensor("wq", [D, D], F32, kind="ExternalInput").ap()
    wk_ext = nc.dram_tensor("wk", [D, D], F32, kind="ExternalInput").ap()

    pooled_ext = nc.dram_tensor("pooled", [L, D], F32, kind="ExternalOutput").ap()
    valid_ext = nc.dram_tensor("valid", [1, L], F32, kind="ExternalOutput").ap()
    stats_ext = nc.dram_tensor("stats", [1, 8], F32, kind="ExternalOutput").ap()
    dbg_ext = nc.dram_tensor("dbg", [4, L], F32, kind="ExternalOutput").ap()
    # DRAM scratch (ExternalOutput: internal DRAM tensors fail NEFF load here)
    scr_h = nc.dram_tensor("scr_h", [1, L], F32, kind="ExternalOutput").ap()
    scr_a1 = nc.dram_tensor("scr_a1", [1, L], F32, kind="ExternalOutput").ap()
    scr_len = nc.dram_tensor("scr_len", [1, L], F32, kind="ExternalOutput").ap()
    scr_i0 = nc.dram_tensor("scr_i0", [16, 256], I16, kind="ExternalOutput").ap()
    scr_i1 = nc.dram_tensor("scr_i1", [16, 256], I16, kind="ExternalOutput").ap()

    with tile.TileContext(nc) as tc:
        with (
            tc.tile_pool(name="const", bufs=1) as const,
            tc.tile_pool(name="keep", bufs=1) as keep,
            tc.tile_pool(name="stream", bufs=2) as stream,
            tc.tile_pool(name="ps_tt", bufs=2, space="PSUM") as ps_tt,
            tc.tile_pool(name="ps_proj", bufs=2, space="PSUM") as ps_proj,
            tc.tile_pool(name="ps_g", bufs=2, space="PSUM") as ps_g,
        ):
            # ---------------- constants ----------------
            ident = const.tile([128, 128], F32, tag="ident")
            masks.make_identity(nc, ident[:])
            zeros = const.tile([128, 128], F32, tag="zeros")
            nc.vector.memset(zeros[:], 0.0)
            ones_col = const.tile([128, 1], F32, tag="ones_col")
            nc.vector.memset(ones_col[:], 1.0)
            ones_row = const.tile([1, 128], F32, tag="ones_row")
            nc.vector.memset(ones_row[:], 1.0)
            iw_f = const.tile([16, 256], F32, tag="iw_f")
            nc.gpsimd.iota(iw_f[:], pattern=[[16, 256]], base=0,
                           channel_multiplier=1,
                           allow_small_or_imprecise_dtypes=True)
            i1232_f = const.tile([128, 32], F32, tag="i1232_f")
            nc.gpsimd.iota(i1232_f[:], pattern=[[128, 32]], base=0,
                           channel_multiplier=1,
                           allow_small_or_imprecise_dtypes=True)

            # persistent small results
            scal = keep.tile([1, 64], F32, tag="scal")  # 0:counts 1:msum 2:nf0 3:nf1
            ssq_sb = keep.tile([128, NTT], F32, tag="ssq_sb")
            counts_b128 = keep.tile([128, 1], F32, tag="counts_b128")
            idx0 = keep.tile([128, 256], I16, tag="idx0")
            idx1 = keep.tile([128, 256], I16, tag="idx1")
            scale128 = keep.tile([128, 32], F32, tag="scale128")
            len128 = keep.tile([128, 32], F32, tag="len128")
            valid128 = keep.tile([128, 32], F32, tag="valid128")
            a0w = keep.tile([16, 256], F32, tag="a0w")
            a1w = keep.tile([16, 256], F32, tag="a1w")
            list0 = keep.tile([16, 256], F32, tag="list0")
            list1 = keep.tile([16, 256], F32, tag="list1")
            vsw = keep.tile([16, 256], F32, tag="vsw")
            len_w = keep.tile([16, 256], F32, tag="len_w")
            i0_16 = keep.tile([16, 256], I16, tag="i0_16")
            i1_16 = keep.tile([16, 256], I16, tag="i1_16")
            nf0 = keep.tile([1, 1], U32, tag="nf0")
            nf1 = keep.tile([1, 1], U32, tag="nf1")

            with tc.tile_pool(name="mpool", bufs=1) as mpool:
                m_sb = mpool.tile([128, NDC, D], F32, tag="m_sb")

                # ---------------- phase 0: M = Wq^T @ Wk ----------------
                with tc.tile_pool(name="wkp", bufs=8) as wkp:
                    wk_t = [wkp.tile([128, D], F32, tag="wk", name=f"wk{i}")
                            for i in range(8)]
                    for i in range(8):
                        nc.gpsimd.dma_start(
                            out=wk_t[i][:],
                            in_=wk_ext[128 * i:128 * i + 128, :])
                    for dd in range(8):
                        wq_blocks = []
                        for ih in range(8):
                            wq_blk = stream.tile([128, 128], F32, tag="wqs",
                                                 name=f"wqb{dd}_{ih}", bufs=9)
                            nc.gpsimd.dma_start(
                                out=wq_blk[:],
                                in_=wq_ext[128 * ih:128 * ih + 128,
                                           128 * dd:128 * dd + 128])
                            wq_blocks.append(wq_blk)
                        for eh in range(2):
                            pm = ps_proj.tile([128, 512], F32, tag="pp",
                                              name=f"pm{dd}_{eh}")
                            for ih in range(8):
                                nc.tensor.matmul(
                                    pm[:], wq_blocks[ih][:],
                                    wk_t[ih][:, 512 * eh:512 * eh + 512],
                                    start=(ih == 0), stop=(ih == 7))
                            nc.vector.tensor_copy(
                                m_sb[:, dd, 512 * eh:512 * eh + 512], pm[:])

                with tc.tile_pool(name="gpool", bufs=1) as gpool:
                    g_row = gpool.tile([1, L], F32, tag="g_row")

                    # ------------ phase A: transpose chunks + proj + G ------------
                    with tc.tile_pool(name="apool", bufs=1) as apool:
                        htc = [apool.tile([128, NDC, LCH + 1], F32, tag="htc",
                                          name=f"htc{c}", bufs=2)
                               for c in range(NLC)]
                        for c in range(NLC):
                            for tt in range(4):
                                t = 4 * c + tt
                                htile = stream.tile([128, D], F32, tag="htile",
                                                    name=f"ha{t}")
                                nc.gpsimd.dma_start(
                                    out=htile[:],
                                    in_=h_ext[128 * t:128 * t + 128, :])
                                sq = apool.tile([128, D], F32, tag="sqd",
                                                name=f"sq{t}", bufs=2)
                                nc.scalar.activation(
                                    sq[:], htile[:], Act.Square,
                                    accum_out=ssq_sb[:, t:t + 1])
                                ptt = ps_tt.tile([128, NDC, 128], F32,
                                                 tag="ptt", name=f"ptt{t}")
                                for dc in range(NDC):
                                    nc.tensor.transpose(
                                        ptt[:, dc, :],
                                        htile[:, 128 * dc:128 * dc + 128],
                                        ident[:])
                                nc.scalar.copy(
                                    htc[c][:, :, 128 * tt:128 * tt + 128],
                                    ptt[:])
                                if tt == 0 and c > 0:
                                    nc.scalar.copy(
                                        htc[c - 1][:, :, LCH:LCH + 1],
                                        ptt[:, :, 0:1])

                            # projection for chunk c (needs col LCH from c+1,
                            # so chunk c's proj is emitted after c+1's first
                            # tile; tile framework orders by data deps anyway)
                        for c in range(NLC):
                            width = LCH if c < NLC - 1 else LCH - 1
                            pg = ps_g.tile([1, LCH], F32, tag="pg",
                                           name=f"pg{c}")
                            for e in range(NDC):
                                pp = ps_proj.tile([128, LCH], F32, tag="pp",
                                                  name=f"pp{c}_{e}")
                                for d in range(NDC):
                                    nc.tensor.matmul(
                                        pp[:],
                                        m_sb[:, d, 128 * e:128 * e + 128],
                                        htc[c][:, d, 0:LCH],
                                        start=(d == 0), stop=(d == NDC - 1))
                                ph = apool.tile([128, LCH], F32, tag="ph",
                                                name=f"ph{c}_{e}", bufs=2)
                                nc.vector.tensor_tensor(
                                    ph[:, :width], pp[:, :width],
                                    htc[c][:, e, 1:1 + width], Alu.mult)
                                nc.tensor.matmul(
                                    pg[:, :width], ones_col[:], ph[:, :width],
                                    start=(e == 0), stop=(e == NDC - 1))
                            nc.scalar.copy(g_row[0:1, LCH * c:LCH * c + width],
                                           pg[:, :width])
                        nc.vector.memset(g_row[0:1, L - 1:L], 0.0)

                    # ---------------- phase R: rows ----------------
                    with tc.tile_pool(name="rpool", bufs=1) as rpool:
                        iota_f = rpool.tile([1, L], F32, tag="iota_f")
                        nc.gpsimd.iota(iota_f[:], pattern=[[1, L]], base=0,
                                       channel_multiplier=0,
                                       allow_small_or_imprecise_dtypes=True)
                        rinv = rpool.tile([1, L], F32, tag="rinv")
                        w1 = rpool.tile([1, L], F32, tag="w1")
                        w2 = rpool.tile([1, L], F32, tag="w2")
                        w3 = rpool.tile([1, L], F32, tag="w3")

                        # ssq [128,32] -> flat row (token = 128*t + p)
                        nc.gpsimd.dma_start(
                            out=scr_len[:].rearrange("o (t p) -> (o p) t",
                                                     p=128),
                            in_=ssq_sb[:])
                        nc.gpsimd.dma_start(out=w1[:], in_=scr_len[:])
                        nc.scalar.activation(w1[:], w1[:], Act.Sqrt)
                        nc.vector.reciprocal(rinv[:], w1[:])

                        # cos[j] = G[j-1]*rinv[j-1]*rinv[j], j>=1  (w1 := cos)
                        nc.vector.tensor_tensor(w1[0:1, 1:L],
                                                g_row[0:1, 0:L - 1],
                                                rinv[0:1, 0:L - 1], Alu.mult)
                        nc.vector.tensor_tensor(w1[0:1, 1:L], w1[0:1, 1:L],
                                                rinv[0:1, 1:L], Alu.mult)
                        # u (w2)
                        nc.gpsimd.dma_start(out=w2[:], in_=nz_ext[:])
                        nc.vector.tensor_scalar(w2[:], w2[:], 1e-6, 1.0 - 1e-6,
                                                Alu.max, Alu.min)
                        # m = u - 0.5 - 0.5*cos  (g_row := m)
                        nc.scalar.activation(g_row[:], w1[:], Act.Copy,
                                             scale=-0.5)
                        nc.vector.tensor_scalar(g_row[:], g_row[:], 0.5, None,
                                                Alu.subtract)
                        nc.vector.tensor_tensor(g_row[:], g_row[:], w2[:],
                                                Alu.add)
                        nc.vector.memset(g_row[0:1, 0:1], 1.0)
                        # hard (w1)
                        nc.vector.tensor_scalar(w1[:], g_row[:], 0.0, None,
                                                Alu.is_gt)
                        # mask path (w2 := mask)
                        nc.gpsimd.dma_start(out=w2[:], in_=mk_ext[:])
                        nc.vector.tensor_tensor(w1[:], w1[:], w2[:], Alu.mult)
                        nc.vector.tensor_scalar(w3[:], w2[:], 0.0, None,
                                                Alu.is_equal)
                        nc.vector.tensor_tensor_scan(w3[:], w3[:], w3[:], 0.0,
                                                     Alu.add, Alu.bypass)
                        nc.vector.tensor_scalar(w3[:], w3[:], 1.0, None,
                                                Alu.is_equal)
                        nc.vector.tensor_scalar(rinv[:], w2[:], 0.0, None,
                                                Alu.is_equal)
                        nc.vector.tensor_tensor(w3[:], w3[:], rinv[:],
                                                Alu.mult)
                        # last_real (rinv) = shift-left(w3)
                        nc.vector.memset(rinv[0:1, L - 1:L], 0.0)
                        nc.vector.tensor_copy(rinv[0:1, 0:L - 1],
                                              w3[0:1, 1:L])
                        nc.vector.tensor_tensor(w1[:], w1[:], rinv[:], Alu.max)

                        nc.vector.tensor_reduce(scal[0:1, 0:1], w1[:], X,
                                                Alu.add)
                        nc.vector.tensor_reduce(scal[0:1, 1:2], w2[:], X,
                                                Alu.add)

                        # broadcast counts to 128 partitions (rank-1 matmul)
                        pcb = ps_g.tile([128, 1], F32, tag="pg", name="pcb")
                        nc.tensor.matmul(pcb[:], ones_row[:], scal[0:1, 0:1],
                                         start=True, stop=True)
                        nc.vector.tensor_copy(counts_b128[:], pcb[:])

                        # valid (wrapped) -> DMA out
                        nc.vector.tensor_scalar(vsw[:], iw_f[:],
                                                counts_b128[0:16, 0:1], None,
                                                Alu.is_lt)
                        nc.gpsimd.dma_start(
                            out=valid_ext[:].rearrange("o (f p) -> (o p) f",
                                                       p=16),
                            in_=vsw[:])

                        # A0 = hard*(iota+1) - 1 -> wrapped
                        nc.vector.tensor_scalar(w3[:], iota_f[:], 1.0, None,
                                                Alu.add)
                        nc.vector.tensor_tensor(w3[:], w3[:], w1[:], Alu.mult)
                        nc.vector.tensor_scalar(w3[:], w3[:], 1.0, None,
                                                Alu.subtract)
                        nc.gpsimd.dma_start(out=scr_h[:], in_=w3[:])
                        nc.gpsimd.dma_start(
                            out=a0w[:],
                            in_=scr_h[:].rearrange("o (f p) -> (o p) f", p=16))
                        # A1[j] = hard[j+1]*(j+2)-1, j<4095 ; A1[4095]=4096
                        nc.vector.tensor_scalar(w3[0:1, 0:L - 1],
                                                iota_f[0:1, 0:L - 1], 2.0,
                                                None, Alu.add)
                        nc.vector.tensor_tensor(w3[0:1, 0:L - 1],
                                                w3[0:1, 0:L - 1],
                                                w1[0:1, 1:L], Alu.mult)
                        nc.vector.tensor_scalar(w3[0:1, 0:L - 1],
                                                w3[0:1, 0:L - 1], 1.0, None,
                                                Alu.subtract)
                        nc.vector.memset(w3[0:1, L - 1:L], float(L))
                        nc.gpsimd.dma_start(out=scr_a1[:], in_=w3[:])
                        nc.gpsimd.dma_start(
                            out=a1w[:],
                            in_=scr_a1[:].rearrange("o (f p) -> (o p) f",
                                                    p=16))

                        # debug rows: m and hard
                        nc.gpsimd.dma_start(out=dbg_ext[0:1, :], in_=g_row[:])
                        nc.gpsimd.dma_start(out=dbg_ext[1:2, :], in_=w1[:])

                        # boundary lists
                        nc.gpsimd.sparse_gather(list0[:], a0w[:],
                                                num_found=nf0[:])
                        nc.gpsimd.sparse_gather(list1[:], a1w[:],
                                                num_found=nf1[:])
                        nc.vector.tensor_copy(scal[0:1, 2:3], nf0[:])
                        nc.vector.tensor_copy(scal[0:1, 3:4], nf1[:])
                        nc.gpsimd.dma_start(out=stats_ext[:],
                                            in_=scal[0:1, 0:8])

                        # dummy entries (s >= counts) -> 0
                        nc.vector.tensor_tensor(list0[:], list0[:], vsw[:],
                                                Alu.mult)
                        nc.vector.tensor_tensor(list1[:], list1[:], vsw[:],
                                                Alu.mult)

                        # len -> scale [128, 32] (s-order)
                        nc.vector.tensor_tensor(len_w[:], list1[:], list0[:],
                                                Alu.subtract)
                        nc.gpsimd.dma_start(
                            out=scr_len[:].rearrange("o (f p) -> (o p) f",
                                                     p=16),
                            in_=len_w[:])
                        nc.gpsimd.dma_start(
                            out=len128[:],
                            in_=scr_len[:].rearrange("o (c p) -> (o p) c",
                                                     p=128))
                        nc.vector.tensor_scalar(len128[:], len128[:], 1.0,
                                                None, Alu.max)
                        nc.vector.reciprocal(scale128[:], len128[:])
                        nc.vector.tensor_scalar(valid128[:], i1232_f[:],
                                                counts_b128[:, 0:1], None,
                                                Alu.is_lt)
                        nc.vector.tensor_tensor(scale128[:], scale128[:],
                                                valid128[:], Alu.mult)

                        # idx lists -> int16, replicated across 16-part groups
                        nc.vector.tensor_copy(i0_16[:], list0[:])
                        nc.vector.tensor_copy(i1_16[:], list1[:])
                        nc.gpsimd.dma_start(out=scr_i0[:], in_=i0_16[:])
                        nc.gpsimd.dma_start(out=scr_i1[:], in_=i1_16[:])
                        for g in range(8):
                            nc.gpsimd.dma_start(
                                out=idx0[16 * g:16 * g + 16, :], in_=scr_i0[:])
                            nc.gpsimd.dma_start(
                                out=idx1[16 * g:16 * g + 16, :], in_=scr_i1[:])

            # ---------- phase B: prefix sums + gather + pooled ----------
            with tc.tile_pool(name="bpool", bufs=1) as bpool:
                csum = bpool.tile([128, NDC, L + 1], F32, tag="csum")
                nc.vector.memset(csum[:, :, 0:1], 0.0)
                for t in range(NTT):
                    htile = stream.tile([128, D], F32, tag="htile",
                                        name=f"hb{t}")
                    nc.gpsimd.dma_start(out=htile[:],
                                        in_=h_ext[128 * t:128 * t + 128, :])
                    ptt = ps_tt.tile([128, NDC, 128], F32, tag="ptt",
                                     name=f"ptb{t}")
                    for dc in range(NDC):
                        nc.tensor.transpose(ptt[:, dc, :],
                                            htile[:, 128 * dc:128 * dc + 128],
                                            ident[:])
                    for dc in range(NDC):
                        nc.vector.tensor_tensor_scan(
                            csum[:, dc, 1 + 128 * t:129 + 128 * t],
                            ptt[:, dc, :],
                            zeros[:],
                            csum[:, dc, 128 * t:128 * t + 1],
                            Alu.add, Alu.bypass)

                for sc in range(8):
                    diff = bpool.tile([128, NDC, LCH], F32, tag="diff",
                                      name=f"diff{sc}", bufs=2)
                    for dc in range(NDC):
                        src3 = csum[:, dc, :].unsqueeze(2)
                        g1v = diff[:, dc, :].unsqueeze(2)
                        nc.gpsimd.ap_gather(g1v, src3,
                                            idx1[:, 32 * sc:32 * sc + 32],
                                            channels=128, num_elems=L + 1,
                                            d=1, num_idxs=LCH)
                        g0 = bpool.tile([128, LCH, 1], F32, tag="g0",
                                        name=f"g0_{sc}_{dc}", bufs=2)
                        nc.gpsimd.ap_gather(g0[:], src3,
                                            idx0[:, 32 * sc:32 * sc + 32],
                                            channels=128, num_elems=L + 1,
                                            d=1, num_idxs=LCH)
                        nc.vector.tensor_tensor(diff[:, dc, :],
                                                diff[:, dc, :],
                                                g0[:, :, 0], Alu.subtract)
                    for sb in range(4):
                        s0 = 128 * sb
                        blk = 4 * sc + sb
                        ptb = ps_tt.tile([128, NDC, 128], F32, tag="ptt",
                                         name=f"ptp{blk}")
                        for dc in range(NDC):
                            nc.tensor.transpose(ptb[:, dc, :],
                                                diff[:, dc, s0:s0 + 128],
                                                ident[:])
                        pooled_sb = bpool.tile([128, D], F32, tag="pooled_sb",
                                               name=f"po{blk}", bufs=2)
                        nc.scalar.activation(
                            pooled_sb[:],
                            ptb[:].rearrange("p dc l -> p (dc l)"),
                            Act.Copy, scale=scale128[:, blk:blk + 1])
                        nc.gpsimd.dma_start(
                            out=pooled_ext[128 * blk:128 * blk + 128, :],
                            in_=pooled_sb[:])

    nc.compile()
    return nc


def _get_runner():
    if "runner" in _CACHE:
        return _CACHE["runner"]

    import jax
    import numpy as _np
    import concourse.mybir as mybir
    from concourse import bass2jax
    from concourse.bass2jax import _bass_exec_p, partition_id_tensor
    from jax.sharding import Mesh, PartitionSpec
    from jax.experimental.shard_map import shard_map

    nc = build_nc()
    bass2jax.install_neuronx_cc_hook()

    partition_name = (nc.partition_id_tensor.name if nc.partition_id_tensor
                      else None)
    in_names, out_names, out_avals, zero_outs = [], [], [], []
    for alloc in nc.m.functions[0].allocations:
        if not isinstance(alloc, mybir.MemoryLocationSet):
            continue
        name = alloc.memorylocations[0].name
        if alloc.kind == "ExternalInput":
            if name != partition_name:
                in_names.append(name)
        elif alloc.kind == "ExternalOutput":
            out_names.append(name)
            shape = tuple(alloc.tensor_shape)
            dtype = mybir.dt.np(alloc.dtype)
            out_avals.append(jax.core.ShapedArray(shape, dtype))
            zero_outs.append(_np.zeros(shape, dtype))
    n_params = len(in_names)
    all_in_names = in_names + out_names
    if partition_name is not None:
        all_in_names.append(partition_name)

    def _body(*args):
        operands = list(args)
        if partition_name is not None:
            operands.append(partition_id_tensor())
        outs = _bass_exec_p.bind(
            *operands,
            out_avals=tuple(out_avals),
            in_names=tuple(all_in_names),
            out_names=tuple(out_names),
            lowering_input_output_aliases=(),
            sim_require_finite=True,
            sim_require_nnan=True,
            nc=nc,
        )
        return tuple(outs)

    devices = jax.devices()[:NCORES]
    mesh = Mesh(np.asarray(devices), ("core",))
    in_specs = (PartitionSpec("core"),) * (n_params + len(out_names))
    out_specs = (PartitionSpec("core"),) * len(out_names)
    sharded = jax.jit(
        shard_map(_body, mesh=mesh, in_specs=in_specs, out_specs=out_specs,
                  check_rep=False),
        keep_unused=True,
    )
    concat_zeros = [
        _np.zeros((NCORES * z.shape[0], *z.shape[1:]), z.dtype)
        for z in zero_outs
    ]

    runner = {
        "nc": nc, "in_names": in_names, "out_names": out_names,
        "out_avals": out_avals, "sharded": sharded,
        "concat_zeros": concat_zeros,
    }
    _CACHE["runner"] = runner
    return runner


def run_cores(in_maps):
    """in_maps: list of NCORES dicts name->np.ndarray. Returns list of dicts."""
    r = _get_runner()
    concat_in = [
        np.concatenate([np.asarray(in_maps[c][n]) for c in range(NCORES)],
                       axis=0)
        for n in r["in_names"]
    ]
    outs = r["sharded"](*concat_in, *r["concat_zeros"])
    outs = [np.asarray(o) for o in outs]
    return [
        {
            n: outs[i].reshape(NCORES, *r["out_avals"][i].shape)[c]
            for i, n in enumerate(r["out_names"])
        }
        for c in range(NCORES)
    ]


def make_in_maps(hidden, attention_mask, noise, Wq, Wk):
    hidden = np.ascontiguousarray(np.asarray(hidden, dtype=np.float32))
    attention_mask = np.asarray(attention_mask, dtype=np.float32)
    noise = np.asarray(noise, dtype=np.float32)
    Wq = np.ascontiguousarray(np.asarray(Wq, dtype=np.float32))
    Wk = np.ascontiguousarray(np.asarray(Wk, dtype=np.float32))
    return [
        {
            "h": hidden[b],
            "nz": noise[b:b + 1],
            "mk": attention_mask[b:b + 1],
            "wq": Wq,
            "wk": Wk,
        }
        for b in range(B)
    ]


def _gammaln_f32(x):
    try:
        import jax.numpy as jnp
        from jax.scipy.special import gammaln
        return float(gammaln(jnp.float32(x)))
    except Exception:
        return float(math.lgamma(float(x)))


def _loss_from_counts(num_boundaries, total_positions):
    k = np.float32(num_boundaries)
    n = np.float32(total_positions)
    p = np.float32(PRIOR)
    log_prob = (
        np.float32(_gammaln_f32(n + 1.0))
        - np.float32(_gammaln_f32(k + 1.0))
        - np.float32(_gammaln_f32(n - k + 1.0))
        + k * np.float32(np.log(p))
        + (n - k) * np.float32(np.log1p(-p))
    )
    return np.float32(-log_prob / np.float32(64.0 ** 2))


def kernel(hidden, attention_mask, noise, Wq, Wk):
    in_maps = make_in_maps(hidden, attention_mask, noise, Wq, Wk)
    results = run_cores(in_maps)
    pooled = np.stack([results[b]["pooled"] for b in range(B)])
    valid = np.stack([results[b]["valid"][0] for b in range(B)])
    counts = np.array([results[b]["stats"][0, 0] for b in range(B)],
                      np.float32)
    msums = np.array([results[b]["stats"][0, 1] for b in range(B)],
                     np.float32)
    num_boundaries = np.float32(counts.sum())
    total_positions = np.float32(msums.sum())
    loss = _loss_from_counts(num_boundaries, total_positions)
    return (pooled, loss, num_boundaries, total_positions, valid)


# revision 6
# speedup vs baseline: 41.4909x; 41.4909x over previous
"""Trainium2 Bass kernel for nn_BoundaryPredictor2 (ragged_sequence).

Data-parallel over batch: one NeuronCore per batch row (B=8, 8 cores).

Per-core pipeline (one batch row, L=4096 tokens, D=1024):
  cos_sim(j) = n_j^T (Wq^T Wk) n_{j+1}  with n_l = h_l / |h_l|
    -> M = Wq^T @ Wk once per core (PE); H transposed chunk-wise on PE;
       P^T = M^T H^T; G[j] = sum_e P^T[e,j] H^T[e,j+1] (DVE mult + PE
       ones-reduce).  Normalization folded into row scalars.
  boundary decision: sigmoid(logit(p)+logit(u)) > 0.5  <=>  p + u > 1
    (p = (1-cos)/2, u = clipped noise) -- no transcendentals needed.
  segment-mean pooling: inclusive prefix sums along tokens (DVE scan over a
    second transposed pass), boundary-position lists via iota+sparse_gather,
    column gathers via ap_gather, per-segment 1/len scale fused into the
    PSUM->SBUF copy after PE transpose-back.
"""

import math

import numpy as np

L = 4096
D = 1024
B = 8
NCORES = 8
LCH = 512            # L-chunk for projection / gathers
NLC = L // LCH       # 8
NTT = L // 128       # 32 token tiles
NDC = D // 128       # 8 d-chunks
PRIOR = 0.2

_CACHE = {}


def build_nc():
    import concourse.bacc as bacc
    import concourse.mybir as mybir
    from concourse import masks, tile

    F32 = mybir.dt.float32
    I16 = mybir.dt.int16
    U32 = mybir.dt.uint32
    Alu = mybir.AluOpType
    Act = mybir.ActivationFunctionType
    X = mybir.AxisListType.X

    nc = bacc.Bacc("TRN2", target_bir_lowering=False, debug=False,
                   num_devices=NCORES)

    h_ext = nc.dram_tensor("h", [L, D], F32, kind="ExternalInput").ap()
    nz_ext = nc.dram_tensor("nz", [1, L], F32, kind="ExternalInput").ap()
    mk_ext = nc.dram_tensor("mk", [1, L], F32, kind="ExternalInput").ap()
    wq_ext = nc.dram_tensor("wq", [D, D], F32, kind="ExternalInput").ap()
    wk_ext = nc.dram_tensor("wk", [D, D], F32, kind="ExternalInput").ap()

    pooled_ext = nc.dram_tensor("pooled", [L, D], F32, kind="ExternalOutput").ap()
    valid_ext = nc.dram_tensor("valid", [1, L], F32, kind="ExternalOutput").ap()
    stats_ext = nc.dram_tensor("stats", [1, 8], F32, kind="ExternalOutput").ap()
    dbg_ext = nc.dram_tensor("dbg", [4, L], F32, kind="ExternalOutput").ap()
    # DRAM scratch (ExternalOutput: internal DRAM tensors fail NEFF load here)
    scr_h = nc.dram_tensor("scr_h", [1, L], F32, kind="ExternalOutput").ap()
    scr_a1 = nc.dram_tensor("scr_a1", [1, L], F32, kind="ExternalOutput").ap()
    scr_len = nc.dram_tensor("scr_len", [1, L], F32, kind="ExternalOutput").ap()
    scr_i1 = nc.dram_tensor("scr_i1", [16, 256], I16, kind="ExternalOutput").ap()

    with tile.TileContext(nc) as tc:
        with (
            tc.tile_pool(name="const", bufs=1) as const,
            tc.tile_pool(name="keep", bufs=1) as keep,
            tc.tile_pool(name="stream", bufs=2) as stream,
            tc.tile_pool(name="ps_tt", bufs=2, space="PSUM") as ps_tt,
            tc.tile_pool(name="ps_proj", bufs=2, space="PSUM") as ps_proj,
            tc.tile_pool(name="ps_g", bufs=2, space="PSUM") as ps_g,
        ):
            # ---------------- constants ----------------
            ident = const.tile([128, 128], F32, tag="ident")
            masks.make_identity(nc, ident[:])
            zeros = const.tile([128, 128], F32, tag="zeros")
            nc.vector.memset(zeros[:], 0.0)
            ones_col = const.tile([128, 1], F32, tag="ones_col")
            nc.vector.memset(ones_col[:], 1.0)
            ones_row = const.tile([1, 128], F32, tag="ones_row")
            nc.vector.memset(ones_row[:], 1.0)
            iw_f = const.tile([16, 256], F32, tag="iw_f")
            nc.gpsimd.iota(iw_f[:], pattern=[[16, 256]], base=0,
                           channel_multiplier=1,
                           allow_small_or_imprecise_dtypes=True)
            i1232_f = const.tile([128, 32], F32, tag="i1232_f")
            nc.gpsimd.iota(i1232_f[:], pattern=[[128, 32]], base=0,
                           channel_multiplier=1,
                           allow_small_or_imprecise_dtypes=True)

            # persistent small results
            scal = keep.tile([1, 64], F32, tag="scal")  # 0:counts 1:msum 2:nf0 3:nf1
            ssq_sb = keep.tile([128, NTT], F32, tag="ssq_sb")
            counts_b128 = keep.tile([128, 1], F32, tag="counts_b128")
            idx1 = keep.tile([128, 256], I16, tag="idx1")
            sv_col = keep.tile([128, NDC, 1], F32, tag="sv_col")
            scale128 = keep.tile([128, 32], F32, tag="scale128")
            len128 = keep.tile([128, 32], F32, tag="len128")
            valid128 = keep.tile([128, 32], F32, tag="valid128")
            a0w = keep.tile([16, 256], F32, tag="a0w")
            a1w = keep.tile([16, 256], F32, tag="a1w")
            list0 = keep.tile([16, 256], F32, tag="list0")
            list1 = keep.tile([16, 256], F32, tag="list1")
            vsw = keep.tile([16, 256], F32, tag="vsw")
            len_w = keep.tile([16, 256], F32, tag="len_w")
            i1_16 = keep.tile([16, 256], I16, tag="i1_16")
            nf0 = keep.tile([1, 1], U32, tag="nf0")
            nf1 = keep.tile([1, 1], U32, tag="nf1")

            with tc.tile_pool(name="mpool", bufs=1) as mpool:
                m_sb = mpool.tile([128, NDC, D], F32, tag="m_sb")

                # ---------------- phase 0: M = Wq^T @ Wk ----------------
                with tc.tile_pool(name="wkp", bufs=8) as wkp:
                    wk_t = [wkp.tile([128, D], F32, tag="wk", name=f"wk{i}")
                            for i in range(8)]
                    for i in range(8):
                        nc.sync.dma_start(
                            out=wk_t[i][:],
                            in_=wk_ext[128 * i:128 * i + 128, :])
                    for dd in range(8):
                        wq_blocks = []
                        for ih in range(8):
                            wq_blk = stream.tile([128, 128], F32, tag="wqs",
                                                 name=f"wqb{dd}_{ih}", bufs=9)
                            nc.sync.dma_start(
                                out=wq_blk[:],
                                in_=wq_ext[128 * ih:128 * ih + 128,
                                           128 * dd:128 * dd + 128])
                            wq_blocks.append(wq_blk)
                        for eh in range(2):
                            pm = ps_proj.tile([128, 512], F32, tag="pp",
                                              name=f"pm{dd}_{eh}")
                            for ih in range(8):
                                nc.tensor.matmul(
                                    pm[:], wq_blocks[ih][:],
                                    wk_t[ih][:, 512 * eh:512 * eh + 512],
                                    start=(ih == 0), stop=(ih == 7))
                            nc.vector.tensor_copy(
                                m_sb[:, dd, 512 * eh:512 * eh + 512], pm[:])

                with tc.tile_pool(name="gpool", bufs=1) as gpool:
                    g_row = gpool.tile([1, L], F32, tag="g_row")

                    # ------------ phase A: transpose chunks + proj + G ------------
                    with tc.tile_pool(name="apool", bufs=1) as apool:
                        htc = [apool.tile([128, NDC, LCH + 1], F32, tag="htc",
                                          name=f"htc{c}", bufs=2)
                               for c in range(NLC)]
                        for c in range(NLC):
                            for tt in range(4):
                                t = 4 * c + tt
                                htile = stream.tile([128, D], F32, tag="htile",
                                                    name=f"ha{t}")
                                nc.sync.dma_start(
                                    out=htile[:],
                                    in_=h_ext[128 * t:128 * t + 128, :])
                                sq = apool.tile([128, D], F32, tag="sqd",
                                                name=f"sq{t}", bufs=2)
                                nc.scalar.activation(
                                    sq[:], htile[:], Act.Square,
                                    accum_out=ssq_sb[:, t:t + 1])
                                ptt = ps_tt.tile([128, NDC, 128], F32,
                                                 tag="ptt", name=f"ptt{t}")
                                for dc in range(NDC):
                                    nc.tensor.transpose(
                                        ptt[:, dc, :],
                                        htile[:, 128 * dc:128 * dc + 128],
                                        ident[:])
                                nc.scalar.copy(
                                    htc[c][:, :, 128 * tt:128 * tt + 128],
                                    ptt[:])
                                if tt == 0 and c > 0:
                                    nc.scalar.copy(
                                        htc[c - 1][:, :, LCH:LCH + 1],
                                        ptt[:, :, 0:1])

                            # projection for chunk c (needs col LCH from c+1,
                            # so chunk c's proj is emitted after c+1's first
                            # tile; tile framework orders by data deps anyway)
                        for c in range(NLC):
                            width = LCH if c < NLC - 1 else LCH - 1
                            pg = ps_g.tile([1, LCH], F32, tag="pg",
                                           name=f"pg{c}")
                            for e in range(NDC):
                                pp = ps_proj.tile([128, LCH], F32, tag="pp",
                                                  name=f"pp{c}_{e}")
                                for d in range(NDC):
                                    nc.tensor.matmul(
                                        pp[:],
                                        m_sb[:, d, 128 * e:128 * e + 128],
                                        htc[c][:, d, 0:LCH],
                                        start=(d == 0), stop=(d == NDC - 1))
                                ph = apool.tile([128, LCH], F32, tag="ph",
                                                name=f"ph{c}_{e}", bufs=2)
                                nc.vector.tensor_tensor(
                                    ph[:, :width], pp[:, :width],
                                    htc[c][:, e, 1:1 + width], Alu.mult)
                                nc.tensor.matmul(
                                    pg[:, :width], ones_col[:], ph[:, :width],
                                    start=(e == 0), stop=(e == NDC - 1))
                            nc.scalar.copy(g_row[0:1, LCH * c:LCH * c + width],
                                           pg[:, :width])
                        nc.vector.memset(g_row[0:1, L - 1:L], 0.0)

                    # ---------------- phase R: rows ----------------
                    with tc.tile_pool(name="rpool", bufs=1) as rpool:
                        iota_f = rpool.tile([1, L], F32, tag="iota_f")
                        nc.gpsimd.iota(iota_f[:], pattern=[[1, L]], base=0,
                                       channel_multiplier=0,
                                       allow_small_or_imprecise_dtypes=True)
                        rinv = rpool.tile([1, L], F32, tag="rinv")
                        w1 = rpool.tile([1, L], F32, tag="w1")
                        w2 = rpool.tile([1, L], F32, tag="w2")
                        w3 = rpool.tile([1, L], F32, tag="w3")

                        # ssq [128,32] -> flat row (token = 128*t + p)
                        nc.sync.dma_start(
                            out=scr_len[:].rearrange("o (t p) -> (o p) t",
                                                     p=128),
                            in_=ssq_sb[:])
                        nc.sync.dma_start(out=w1[:], in_=scr_len[:])
                        nc.scalar.activation(w1[:], w1[:], Act.Sqrt)
                        nc.vector.reciprocal(rinv[:], w1[:])

                        # cos[j] = G[j-1]*rinv[j-1]*rinv[j], j>=1  (w1 := cos)
                        nc.vector.tensor_tensor(w1[0:1, 1:L],
                                                g_row[0:1, 0:L - 1],
                                                rinv[0:1, 0:L - 1], Alu.mult)
                        nc.vector.tensor_tensor(w1[0:1, 1:L], w1[0:1, 1:L],
                                                rinv[0:1, 1:L], Alu.mult)
                        # u (w2)
                        nc.sync.dma_start(out=w2[:], in_=nz_ext[:])
                        nc.vector.tensor_scalar(w2[:], w2[:], 1e-6, 1.0 - 1e-6,
                                                Alu.max, Alu.min)
                        # m = u - 0.5 - 0.5*cos  (g_row := m)
                        nc.scalar.activation(g_row[:], w1[:], Act.Copy,
                                             scale=-0.5)
                        nc.vector.tensor_scalar(g_row[:], g_row[:], 0.5, None,
                                                Alu.subtract)
                        nc.vector.tensor_tensor(g_row[:], g_row[:], w2[:],
                                                Alu.add)
                        nc.vector.memset(g_row[0:1, 0:1], 1.0)
                        # hard (w1)
                        nc.vector.tensor_scalar(w1[:], g_row[:], 0.0, None,
                                                Alu.is_gt)
                        # mask path (w2 := mask)
                        nc.sync.dma_start(out=w2[:], in_=mk_ext[:])
                        nc.vector.tensor_tensor(w1[:], w1[:], w2[:], Alu.mult)
                        nc.vector.tensor_scalar(w3[:], w2[:], 0.0, None,
                                                Alu.is_equal)
                        nc.vector.tensor_tensor_scan(w3[:], w3[:], w3[:], 0.0,
                                                     Alu.add, Alu.bypass)
                        nc.vector.tensor_scalar(w3[:], w3[:], 1.0, None,
                                                Alu.is_equal)
                        nc.vector.tensor_scalar(rinv[:], w2[:], 0.0, None,
                                                Alu.is_equal)
                        nc.vector.tensor_tensor(w3[:], w3[:], rinv[:],
                                                Alu.mult)
                        # last_real (rinv) = shift-left(w3)
                        nc.vector.memset(rinv[0:1, L - 1:L], 0.0)
                        nc.vector.tensor_copy(rinv[0:1, 0:L - 1],
                                              w3[0:1, 1:L])
                        nc.vector.tensor_tensor(w1[:], w1[:], rinv[:], Alu.max)

                        nc.vector.tensor_reduce(scal[0:1, 0:1], w1[:], X,
                                                Alu.add)
                        nc.vector.tensor_reduce(scal[0:1, 1:2], w2[:], X,
                                                Alu.add)

                        # broadcast counts to 128 partitions (rank-1 matmul)
                        pcb = ps_g.tile([128, 1], F32, tag="pg", name="pcb")
                        nc.tensor.matmul(pcb[:], ones_row[:], scal[0:1, 0:1],
                                         start=True, stop=True)
                        nc.vector.tensor_copy(counts_b128[:], pcb[:])

                        # valid (wrapped) -> DMA out
                        nc.vector.tensor_scalar(vsw[:], iw_f[:],
                                                counts_b128[0:16, 0:1], None,
                                                Alu.is_lt)
                        nc.sync.dma_start(
                            out=valid_ext[:].rearrange("o (f p) -> (o p) f",
                                                       p=16),
                            in_=vsw[:])

                        # A0 = hard*(iota+1) - 1 -> wrapped
                        nc.vector.tensor_scalar(w3[:], iota_f[:], 1.0, None,
                                                Alu.add)
                        nc.vector.tensor_tensor(w3[:], w3[:], w1[:], Alu.mult)
                        nc.vector.tensor_scalar(w3[:], w3[:], 1.0, None,
                                                Alu.subtract)
                        nc.sync.dma_start(out=scr_h[:], in_=w3[:])
                        nc.sync.dma_start(
                            out=a0w[:],
                            in_=scr_h[:].rearrange("o (f p) -> (o p) f", p=16))
                        # A1[j] = hard[j+1]*(j+2)-1, j<4095 ; A1[4095]=4096
                        nc.vector.tensor_scalar(w3[0:1, 0:L - 1],
                                                iota_f[0:1, 0:L - 1], 2.0,
                                                None, Alu.add)
                        nc.vector.tensor_tensor(w3[0:1, 0:L - 1],
                                                w3[0:1, 0:L - 1],
                                                w1[0:1, 1:L], Alu.mult)
                        nc.vector.tensor_scalar(w3[0:1, 0:L - 1],
                                                w3[0:1, 0:L - 1], 1.0, None,
                                                Alu.subtract)
                        nc.vector.memset(w3[0:1, L - 1:L], float(L))
                        nc.sync.dma_start(out=scr_a1[:], in_=w3[:])
                        nc.sync.dma_start(
                            out=a1w[:],
                            in_=scr_a1[:].rearrange("o (f p) -> (o p) f",
                                                    p=16))

                        # debug rows: m and hard
                        nc.sync.dma_start(out=dbg_ext[0:1, :], in_=g_row[:])
                        nc.sync.dma_start(out=dbg_ext[1:2, :], in_=w1[:])

                        # boundary lists
                        nc.gpsimd.sparse_gather(list0[:], a0w[:],
                                                num_found=nf0[:])
                        nc.gpsimd.sparse_gather(list1[:], a1w[:],
                                                num_found=nf1[:])
                        nc.vector.tensor_copy(scal[0:1, 2:3], nf0[:])
                        nc.vector.tensor_copy(scal[0:1, 3:4], nf1[:])
                        nc.sync.dma_start(out=stats_ext[:],
                                            in_=scal[0:1, 0:8])

                        # dummy entries (s >= counts) -> 0
                        nc.vector.tensor_tensor(list0[:], list0[:], vsw[:],
                                                Alu.mult)
                        nc.vector.tensor_tensor(list1[:], list1[:], vsw[:],
                                                Alu.mult)

                        # len -> scale [128, 32] (s-order)
                        nc.vector.tensor_tensor(len_w[:], list1[:], list0[:],
                                                Alu.subtract)
                        nc.sync.dma_start(
                            out=scr_len[:].rearrange("o (f p) -> (o p) f",
                                                     p=16),
                            in_=len_w[:])
                        nc.sync.dma_start(
                            out=len128[:],
                            in_=scr_len[:].rearrange("o (c p) -> (o p) c",
                                                     p=128))
                        nc.vector.tensor_scalar(len128[:], len128[:], 1.0,
                                                None, Alu.max)
                        nc.vector.reciprocal(scale128[:], len128[:])
                        nc.vector.tensor_scalar(valid128[:], i1232_f[:],
                                                counts_b128[:, 0:1], None,
                                                Alu.is_lt)
                        nc.vector.tensor_tensor(scale128[:], scale128[:],
                                                valid128[:], Alu.mult)

                        # idx list -> int16, replicated across 16-part groups
                        nc.vector.tensor_copy(i1_16[:], list1[:])
                        nc.sync.dma_start(out=scr_i1[:], in_=i1_16[:])
                        for g in range(8):
                            nc.sync.dma_start(
                                out=idx1[16 * g:16 * g + 16, :], in_=scr_i1[:])

            # ---------- phase B: prefix sums + gather + pooled ----------
            with tc.tile_pool(name="bpool", bufs=1) as bpool:
                csum = bpool.tile([128, NDC, L + 1], F32, tag="csum")
                nc.vector.memset(csum[:, :, 0:1], 0.0)
                for t in range(NTT):
                    htile = stream.tile([128, D], F32, tag="htile",
                                        name=f"hb{t}")
                    nc.sync.dma_start(out=htile[:],
                                        in_=h_ext[128 * t:128 * t + 128, :])
                    ptt = ps_tt.tile([128, NDC, 128], F32, tag="ptt",
                                     name=f"ptb{t}")
                    for dc in range(NDC):
                        nc.tensor.transpose(ptt[:, dc, :],
                                            htile[:, 128 * dc:128 * dc + 128],
                                            ident[:])
                    for dc in range(NDC):
                        nc.vector.tensor_tensor_scan(
                            csum[:, dc, 1 + 128 * t:129 + 128 * t],
                            ptt[:, dc, :],
                            zeros[:],
                            csum[:, dc, 128 * t:128 * t + 1],
                            Alu.add, Alu.bypass)

                for q in range(4):        # 1024 segments per round
                    diffb = bpool.tile([128, NDC, 1024], F32, tag="diffb",
                                       name=f"diffb{q}", bufs=1)
                    for dc in range(NDC):
                        gb = bpool.tile([128, 1024, 1], F32, tag="gb",
                                        name=f"gb{q}_{dc}", bufs=2)
                        nc.gpsimd.ap_gather(gb[:], csum[:, dc, :].unsqueeze(2),
                                            idx1[:, 64 * q:64 * q + 64],
                                            channels=128, num_elems=L + 1,
                                            d=1, num_idxs=1024)
                        # diff[s] = gb[s] - gb[s-1]; boundary col from sv_col
                        nc.vector.tensor_tensor(diffb[:, dc, 1:1024],
                                                gb[:, 1:1024, 0],
                                                gb[:, 0:1023, 0], Alu.subtract)
                        if q == 0:
                            # s=0: minus csum[b_0]=csum[0]=0
                            nc.vector.tensor_copy(diffb[:, dc, 0:1],
                                                  gb[:, 0:1, 0])
                        else:
                            nc.vector.tensor_tensor(diffb[:, dc, 0:1],
                                                    gb[:, 0:1, 0],
                                                    sv_col[:, dc, :],
                                                    Alu.subtract)
                        nc.vector.tensor_copy(sv_col[:, dc, :],
                                              gb[:, 1023:1024, 0])
                    for sb in range(8):
                        s0 = 128 * sb
                        blk = 8 * q + sb
                        ptb = ps_tt.tile([128, NDC, 128], F32, tag="ptt",
                                         name=f"ptp{blk}")
                        for dc in range(NDC):
                            nc.tensor.transpose(ptb[:, dc, :],
                                                diffb[:, dc, s0:s0 + 128],
                                                ident[:])
                        pooled_sb = bpool.tile([128, D], F32, tag="pooled_sb",
                                               name=f"po{blk}", bufs=2)
                        nc.scalar.activation(
                            pooled_sb[:],
                            ptb[:].rearrange("p dc l -> p (dc l)"),
                            Act.Copy, scale=scale128[:, blk:blk + 1])
                        nc.sync.dma_start(
                            out=pooled_ext[128 * blk:128 * blk + 128, :],
                            in_=pooled_sb[:])

    nc.compile()
    return nc


def _get_runner():
    if "runner" in _CACHE:
        return _CACHE["runner"]

    import jax
    import numpy as _np
    import concourse.mybir as mybir
    from concourse import bass2jax
    from concourse.bass2jax import _bass_exec_p, partition_id_tensor
    from jax.sharding import Mesh, PartitionSpec
    from jax.experimental.shard_map import shard_map

    nc = build_nc()
    bass2jax.install_neuronx_cc_hook()

    partition_name = (nc.partition_id_tensor.name if nc.partition_id_tensor
                      else None)
    in_names, out_names, out_avals, zero_outs = [], [], [], []
    for alloc in nc.m.functions[0].allocations:
        if not isinstance(alloc, mybir.MemoryLocationSet):
            continue
        name = alloc.memorylocations[0].name
        if alloc.kind == "ExternalInput":
            if name != partition_name:
                in_names.append(name)
        elif alloc.kind == "ExternalOutput":
            out_names.append(name)
            shape = tuple(alloc.tensor_shape)
            dtype = mybir.dt.np(alloc.dtype)
            out_avals.append(jax.core.ShapedArray(shape, dtype))
            zero_outs.append(_np.zeros(shape, dtype))
    n_params = len(in_names)
    all_in_names = in_names + out_names
    if partition_name is not None:
        all_in_names.append(partition_name)

    def _body(*args):
        operands = list(args)
        if partition_name is not None:
            operands.append(partition_id_tensor())
        outs = _bass_exec_p.bind(
            *operands,
            out_avals=tuple(out_avals),
            in_names=tuple(all_in_names),
            out_names=tuple(out_names),
            lowering_input_output_aliases=(),
            sim_require_finite=True,
            sim_require_nnan=True,
            nc=nc,
        )
        return tuple(outs)

    devices = jax.devices()[:NCORES]
    mesh = Mesh(np.asarray(devices), ("core",))
    in_specs = (PartitionSpec("core"),) * (n_params + len(out_names))
    out_specs = (PartitionSpec("core"),) * len(out_names)
    sharded = jax.jit(
        shard_map(_body, mesh=mesh, in_specs=in_specs, out_specs=out_specs,
                  check_rep=False),
        keep_unused=True,
    )
    concat_zeros = [
        _np.zeros((NCORES * z.shape[0], *z.shape[1:]), z.dtype)
        for z in zero_outs
    ]

    runner = {
        "nc": nc, "in_names": in_names, "out_names": out_names,
        "out_avals": out_avals, "sharded": sharded,
        "concat_zeros": concat_zeros,
    }
    _CACHE["runner"] = runner
    return runner


def run_cores(in_maps):
    """in_maps: list of NCORES dicts name->np.ndarray. Returns list of dicts."""
    r = _get_runner()
    concat_in = [
        np.concatenate([np.asarray(in_maps[c][n]) for c in range(NCORES)],
                       axis=0)
        for n in r["in_names"]
    ]
    outs = r["sharded"](*concat_in, *r["concat_zeros"])
    outs = [np.asarray(o) for o in outs]
    return [
        {
            n: outs[i].reshape(NCORES, *r["out_avals"][i].shape)[c]
            for i, n in enumerate(r["out_names"])
        }
        for c in range(NCORES)
    ]


def make_in_maps(hidden, attention_mask, noise, Wq, Wk):
    hidden = np.ascontiguousarray(np.asarray(hidden, dtype=np.float32))
    attention_mask = np.asarray(attention_mask, dtype=np.float32)
    noise = np.asarray(noise, dtype=np.float32)
    Wq = np.ascontiguousarray(np.asarray(Wq, dtype=np.float32))
    Wk = np.ascontiguousarray(np.asarray(Wk, dtype=np.float32))
    return [
        {
            "h": hidden[b],
            "nz": noise[b:b + 1],
            "mk": attention_mask[b:b + 1],
            "wq": Wq,
            "wk": Wk,
        }
        for b in range(B)
    ]


def _gammaln_f32(x):
    try:
        import jax.numpy as jnp
        from jax.scipy.special import gammaln
        return float(gammaln(jnp.float32(x)))
    except Exception:
        return float(math.lgamma(float(x)))


def _loss_from_counts(num_boundaries, total_positions):
    k = np.float32(num_boundaries)
    n = np.float32(total_positions)
    p = np.float32(PRIOR)
    log_prob = (
        np.float32(_gammaln_f32(n + 1.0))
        - np.float32(_gammaln_f32(k + 1.0))
        - np.float32(_gammaln_f32(n - k + 1.0))
        + k * np.float32(np.log(p))
        + (n - k) * np.float32(np.log1p(-p))
    )
    return np.float32(-log_prob / np.float32(64.0 ** 2))


def kernel(hidden, attention_mask, noise, Wq, Wk):
    in_maps = make_in_maps(hidden, attention_mask, noise, Wq, Wk)
    results = run_cores(in_maps)
    pooled = np.stack([results[b]["pooled"] for b in range(B)])
    valid = np.stack([results[b]["valid"][0] for b in range(B)])
    counts = np.array([results[b]["stats"][0, 0] for b in range(B)],
                      np.float32)
    msums = np.array([results[b]["stats"][0, 1] for b in range(B)],
                     np.float32)
    num_boundaries = np.float32(counts.sum())
    total_positions = np.float32(msums.sum())
    loss = _loss_from_counts(num_boundaries, total_positions)
    return (pooled, loss, num_boundaries, total_positions, valid)
